# revision 25
# baseline (speedup 1.0000x reference)
"""MoE transformer encoder kernel for 8 TRN2 NeuronCores.

Sharding: data-parallel over batch (B=16 -> 2 per core) for the 4 encoder
layers; expert-parallel experts (1 per core, masked-sum AllReduce); head
matmul sharded over vocab columns. Small AllGather of pooled reps.

Layout: activations are feature-major xT[D, tokens] on-chip; weights are
host-pre-transposed and pre-tiled for contiguous DMA. Projection matmuls
run in bf16 (full PE stream rate; fp32/f32r stream at half rate); the
residual stream / LayerNorm stats run in f32r, the gating/expert path in
fp32, and the final layer's LN output stays f32r so the pooled reps that
feed the router are ~1e-4 accurate. LN stats use ones-matmul partition
reductions; LN scale/bias are folded into adjacent weights on host.
"""
import os
import sys

for _p in ("/opt/trn_rl_repo", "/root/.axon_site/_ro/trn_rl_repo"):
    if _p not in sys.path:
        sys.path.append(_p)

import ml_dtypes
import numpy as np
import concourse.bacc as bacc
import concourse.mybir as mybir
from concourse.tile import TileContext
from concourse.bass_utils import run_bass_kernel_spmd

S, B, D, H, L = 512, 16, 1024, 16, 4
DFF, E, V = 2048, 8, 50257
HD = D // H
SCALE = float(1.0 / np.sqrt(HD))
EPS = 1e-5
NC = 8
BC = B // NC          # 2 batch rows per core
T = S * BC            # 1024 tokens per core
VS = 6656             # 13*512 vocab cols per core
VPAD = VS * NC
NKT = D // 128        # 8 contraction tiles over D
NFT = DFF // 128      # 16 tiles over DFF

f32 = mybir.dt.float32
f32r = mybir.dt.float32r
bf16 = mybir.dt.bfloat16
i32 = mybir.dt.int32
u32 = mybir.dt.uint32
AL = mybir.AluOpType
AF = mybir.ActivationFunctionType
AX = mybir.AxisListType

RG = [list(range(NC))]

# SBUF slot budgets (bufs per tag)
BUFS_Z = 14      # bf16 residual-stream tiles (1KB/part each)
BUFS_ZF = 9      # f32r final-layer residual tiles (2KB)
BUFS_M5 = 46     # bf16 transient activations (1KB)
BUFS_UF = 17     # f32r pre-norm residual + squares (2KB)
BUFS_ST = 5
BUFS_WT = 5      # bf16 weight tiles (2KB)
BUFS_WVH = 5     # bf16 (128,8,512) rhs weight blocks (8KB)


def _build_nc():
    nc = bacc.Bacc(num_devices=NC)

    def inp(name, shape, dt=bf16):
        return nc.dram_tensor(name, shape, dt, kind="ExternalInput")

    io = {}
    io["xT"] = inp("xT", (NKT, 128, T), f32r)
    io["xTb"] = inp("xTb", (NKT, 128, T))
    for l in range(L):
        io[f"wqkv{l}"] = inp(f"wqkv{l}", (16, 128, NKT, 128))   # [oi, p, kt, o]
        io[f"wv{l}"] = inp(f"wv{l}", (2, 128, NKT, 512))        # [oc, p, kt, o]
        io[f"bqkc{l}"] = inp(f"bqkc{l}", (2 * D, 1), f32)
        io[f"bvr{l}"] = inp(f"bvr{l}", (1, D))
        io[f"wo{l}"] = inp(f"wo{l}", (NKT, 128, NKT, 128))
        io[f"cbo{l}"] = inp(f"cbo{l}", (1, D))
        io[f"w1{l}"] = inp(f"w1{l}", (NFT, 128, NKT, 128))
        io[f"b1c{l}"] = inp(f"b1c{l}", (DFF, 1), f32)
        io[f"w2{l}"] = inp(f"w2{l}", (NKT, 128, NFT, 128))
        io[f"cb2{l}"] = inp(f"cb2{l}", (1, D))
        io[f"sprev{l}"] = inp(f"sprev{l}", (D, 1), f32)
        io[f"s1_{l}"] = inp(f"s1_{l}", (D, 1), f32)
    io["slmul"] = inp("slmul", (D, 1), f32)
    io["blast"] = inp("blast", (D, 1), f32)
    io["wg"] = inp("wg", (D, E), f32)
    io["bgr"] = inp("bgr", (1, E), f32)
    io["weT"] = inp("weT", (NKT, 128, NKT, 128))
    io["ber"] = inp("ber", (1, D), f32)
    io["ecmp"] = inp("ecmp", (1, 1), f32)
    io["whT"] = inp("whT", (13, 128, NKT, 512))
    io["bhr"] = inp("bhr", (1, VS))
    io["ones_col"] = inp("ones_col", (128, 1), f32r)
    io["ones_row"] = inp("ones_row", (1, 512))
    io["ones16f"] = inp("ones16f", (1, 16), f32)

    logits_s = nc.dram_tensor("logits_s", (16, VS), f32, kind="ExternalOutput")
    gw_out = nc.dram_tensor("gw_out", (16, E), f32, kind="ExternalOutput")
    idx_out = nc.dram_tensor("idx_out", (16, 1), i32, kind="ExternalOutput")

    with TileContext(nc) as tc:
        with tc.tile_pool(name="cn", bufs=1) as cn, \
             tc.tile_pool(name="wp", bufs=1) as wp, \
             tc.tile_pool(name="ap", bufs=1) as ap, \
             tc.tile_pool(name="dr", bufs=1, space="DRAM") as dr, \
             tc.tile_pool(name="pp", bufs=1, space="PSUM") as pp:

            def ptile(name):
                return pp.tile([128, 512], f32, name=name, tag="pb", bufs=8)

            def mm_group(psum_ap, pieces):
                n = len(pieces)
                for i, (lh, rh) in enumerate(pieces):
                    nc.tensor.matmul(psum_ap, lh, rh,
                                     start=(i == 0), stop=(i == n - 1))

            def ztile(name, dt=bf16):
                if dt is f32r:
                    return ap.tile([128, 512], f32r, name=name, tag="zf",
                                   bufs=BUFS_ZF)
                return ap.tile([128, 512], bf16, name=name, tag="z", bufs=BUFS_Z)

            def big(name):     # bf16 transient activation slot
                return ap.tile([128, 512], bf16, name=name, tag="m5", bufs=BUFS_M5)

            def uf_tile(name):  # f32r pre-norm residual / squares
                return ap.tile([128, 512], f32r, name=name, tag="uf", bufs=BUFS_UF)

            def stt_(name):
                return ap.tile([1, 512], f32, name=name, tag="st", bufs=BUFS_ST)

            def wt_tile(name):
                return wp.tile([128, 8, 128], bf16, name=name, tag="wt",
                               bufs=BUFS_WT)

            # layer-0 input: f32r residual copy + host-cast bf16 matmul copy
            zres = [[None] * NKT for _ in range(BC)]
            zmm = [[None] * NKT for _ in range(BC)]
            for b in range(BC):
                for kt in range(NKT):
                    zb = ztile(f"zinb_{b}_{kt}")
                    nc.sync.dma_start(zb[:, :],
                                      io["xTb"][kt, :, b * 512:(b + 1) * 512])
                    zr = ztile(f"zin_{b}_{kt}", f32r)
                    nc.sync.dma_start(zr[:, :],
                                      io["xT"][kt, :, b * 512:(b + 1) * 512])
                    zres[b][kt] = zr
                    zmm[b][kt] = zb

            # ---------------- constants into SBUF
            ones_col = cn.tile([128, 1], f32r, name="ones_col_sb")
            nc.sync.dma_start(ones_col[:, :], io["ones_col"][:, :])
            ones_row = cn.tile([1, 512], bf16, name="ones_row_sb")
            nc.sync.dma_start(ones_row[:, :], io["ones_row"][:, :])
            ones16f = cn.tile([1, 16], f32, name="ones16f_sb")
            nc.sync.dma_start(ones16f[:, :], io["ones16f"][:, :])
            ecmp_sb = cn.tile([1, 1], f32, name="ecmp_sb")
            nc.sync.dma_start(ecmp_sb[:, :], io["ecmp"][:, :])
            ones_col_bf = cn.tile([128, 1], bf16, name="ones_col_bf")
            nc.vector.tensor_copy(ones_col_bf[:, :], ones_col[:, :].bitcast(f32))
            ones16b = cn.tile([1, 16], bf16, name="ones16b")
            nc.vector.tensor_copy(ones16b[:, :], ones16f[:, :])

            lcn = {}
            for l in range(L):
                for nm, src, w in (("bqkc", f"bqkc{l}", 16), ("b1c", f"b1c{l}", 16),
                                   ("sprev", f"sprev{l}", 8), ("s1", f"s1_{l}", 8)):
                    t = cn.tile([128, w], f32, name=f"{nm}{l}_sb")
                    nc.sync.dma_start(
                        t[:, :], io[src][:, :].rearrange("(o p) one -> p (o one)", p=128))
                    lcn[(nm, l)] = t

            def row_const(src_ap, name):
                t = ap.tile([1, D], bf16, name=name, tag="rowc", bufs=3)
                nc.sync.dma_start(t[:, :], src_ap)
                return t

            slmul_sb = cn.tile([128, 8], f32, name="slmul_sb")
            nc.sync.dma_start(slmul_sb[:, :],
                              io["slmul"][:, :].rearrange("(o p) one -> p (o one)", p=128))
            blast_sb = cn.tile([128, 8], f32, name="blast_sb")
            nc.sync.dma_start(blast_sb[:, :],
                              io["blast"][:, :].rearrange("(o p) one -> p (o one)", p=128))
            wg_sb = cn.tile([128, NKT, E], f32, name="wg_sb")
            nc.sync.dma_start(wg_sb[:, :, :],
                              io["wg"][:, :].rearrange("(kt p) e -> p kt e", p=128))
            bgr_sb = cn.tile([1, E], f32, name="bgr_sb")
            nc.sync.dma_start(bgr_sb[:, :], io["bgr"][:, :])
            ber_sb = cn.tile([1, D], f32, name="ber_sb")
            nc.sync.dma_start(ber_sb[:, :], io["ber"][:, :])

            def res_ap(t):
                return t[:, :].bitcast(f32) if t.dtype == f32r else t[:, :]

            def layer_norm(u_tiles, zname, out_dt=bf16):
                """u_tiles: 8 x (128,512) f32r -> 8 normalized z tiles."""
                usq = []
                for kt in range(NKT):
                    sq = big(f"usq_{kt}")
                    if kt % 2 == 0:
                        nc.scalar.activation(sq[:, :], u_tiles[kt][:, :], AF.Square)
                    else:
                        nc.vector.tensor_tensor(
                            sq[:, :], u_tiles[kt][:, :].bitcast(f32),
                            u_tiles[kt][:, :].bitcast(f32), op=AL.mult)
                    usq.append(sq)
                ps_m = ptile("ps_m")
                mm_group(ps_m[0:1, :],
                         [(ones_col[:, :], u_tiles[kt][:, :]) for kt in range(NKT)])
                ps_q = ptile("ps_q")
                mm_group(ps_q[0:1, :],
                         [(ones_col_bf[:, :], usq[kt][:, :]) for kt in range(NKT)])
                mcol = stt_("mcol")
                nc.scalar.mul(mcol[:, :], ps_m[0:1, :], 1.0 / D)
                qcol = stt_("qcol")
                nc.scalar.mul(qcol[:, :], ps_q[0:1, :], 1.0 / D)
                msq = stt_("msq")
                nc.vector.tensor_tensor(msq[:, :], mcol[:, :], mcol[:, :], op=AL.mult)
                var = stt_("var")
                nc.vector.tensor_tensor(var[:, :], qcol[:, :], msq[:, :],
                                        op=AL.subtract)
                vpe = stt_("vpe")
                nc.vector.tensor_single_scalar(vpe[:, :], var[:, :], EPS, op=AL.add)
                rcp = stt_("rcp")
                nc.vector.reciprocal(rcp[:, :], vpe[:, :])
                rstd = stt_("rstd")
                nc.scalar.sqrt(rstd[:, :], rcp[:, :])
                mb = ap.tile([128, 512], f32, name="mb", tag="bc", bufs=2)
                nc.gpsimd.partition_broadcast(mb[:, :], mcol[:, :])
                rb = ap.tile([128, 512], f32, name="rb", tag="bc", bufs=2)
                nc.gpsimd.partition_broadcast(rb[:, :], rstd[:, :])
                zt = []
                for kt in range(NKT):
                    t1 = uf_tile(f"lnt_{kt}")
                    eng = nc.gpsimd if kt % 2 == 0 else nc.vector
                    eng.tensor_tensor(
                        t1[:, :], u_tiles[kt][:, :].bitcast(f32), mb[:, :],
                        op=AL.subtract)
                    zo = ztile(f"{zname}_{kt}", out_dt)
                    eng2 = nc.gpsimd if kt % 2 == 1 else nc.vector
                    eng2.tensor_tensor(
                        zo[:, :], t1[:, :].bitcast(f32), rb[:, :], op=AL.mult)
                    zt.append(zo)
                return zt

            # DRAM bounce tensors for the routing collectives
            agin = dr.tile([D, BC], f32, name="agin")
            agout = dr.tile([NC * D, BC], f32, name="agout", addr_space="Shared")

            # ================= encoder layers =================
            for l in range(L):
                sprev = lcn[("sprev", l)]
                s1 = lcn[("s1", l)]
                bvr = row_const(io[f"bvr{l}"][:, :], f"bvr{l}_sb")
                cbo = row_const(io[f"cbo{l}"][:, :], f"cbo{l}_sb")
                cb2 = row_const(io[f"cb2{l}"][:, :], f"cb2{l}_sb")
                z1 = [None] * BC
                for b in range(BC):
                    zb = zmm[b]
                    scope = nc.named_scope(f"L{l}b{b}_attn")
                    scope.__enter__()
                    # ---- q,k projections (feature-major)
                    qk = []
                    for oi in range(16):
                        wt = wt_tile(f"wqkv_{oi}")
                        nc.sync.dma_start(wt[:, :, :], io[f"wqkv{l}"][oi])
                        ps = ptile("ps_qk")
                        mm_group(ps[:, :],
                                 [(wt[:, kt, :], zb[kt][:, :]) for kt in range(NKT)])
                        qt = big(f"qk_{oi}")
                        nc.scalar.activation(qt[:, :], ps[:, :], AF.Identity,
                                             bias=lcn[("bqkc", l)][:, oi:oi + 1])
                        qk.append(qt)
                    qts, kts = qk[:8], qk[8:]
                    # ---- v (token-major)
                    vts = [[None, None] for _ in range(4)]
                    for oc in range(2):
                        wv = wp.tile([128, NKT, 512], bf16, name=f"wv_{oc}",
                                     tag="wvh", bufs=BUFS_WVH)
                        nc.sync.dma_start(wv[:, :, :], io[f"wv{l}"][oc])
                        for ti in range(4):
                            ps = ptile("ps_v")
                            pieces = [(ones_row[:, 0:128],
                                       bvr[:, oc * 512:oc * 512 + 512])]
                            pieces += [(zb[kt][:, ti * 128:ti * 128 + 128],
                                        wv[:, kt, :]) for kt in range(NKT)]
                            mm_group(ps[:, :], pieces)
                            # 65-wide head blocks: col 64 = ones so the AV
                            # matmul emits the softmax denominator in row 64
                            vt = ap.tile([128, 8, 65], bf16, name=f"v_{ti}_{oc}",
                                         tag="m5", bufs=BUFS_M5)
                            nc.scalar.copy(
                                vt[:, :, 0:64],
                                ps[:, :].rearrange("p (h o) -> p h o", h=8))
                            nc.vector.memset(vt[:, :, 64:65], 1.0)
                            vts[ti][oc] = vt
                    # ---- attention per head (interleaved accumulation keeps
                    # per-head PSUM footprint at ~4 banks -> heads overlap)
                    oT = [big(f"oT_{fi}") for fi in range(NKT)]
                    for h in range(H):
                        fi, ro = h // 2, (h % 2) * 64
                        qh = qts[fi][ro:ro + 64, :]
                        kh = kts[fi][ro:ro + 64, :]
                        oc, hh = h // 8, h % 8
                        ps_av = ptile("ps_av")
                        for kt2 in range(4):
                            ps_s = ptile("ps_s")
                            nc.tensor.matmul(ps_s[:, :],
                                             kh[:, kt2 * 128:kt2 * 128 + 128],
                                             qh[:, :], start=True, stop=True)
                            ex = big(f"exp_{kt2}")
                            nc.scalar.activation(ex[:, :], ps_s[:, :], AF.Exp)
                            nc.tensor.matmul(ps_av[0:65, :],
                                             vts[kt2][oc][:, hh, :],
                                             ex[:, :], start=(kt2 == 0),
                                             stop=(kt2 == 3))
                        rec = stt_("rec")
                        nc.vector.reciprocal(rec[:, :], ps_av[64:65, :])
                        recb = ap.tile([64, 512], f32, name="recb", tag="bc2", bufs=2)
                        nc.gpsimd.partition_broadcast(recb[:, :], rec[:, :])
                        nc.vector.tensor_tensor(oT[fi][ro:ro + 64, :],
                                                ps_av[0:64, :], recb[:, :],
                                                op=AL.mult)
                    # ---- Wo projection + residual -> u1 -> LN1
                    u1 = []
                    for oi in range(NKT):
                        wt = wt_tile(f"wo_{oi}")
                        nc.sync.dma_start(wt[:, :, :], io[f"wo{l}"][oi])
                        ps = ptile("ps_wo")
                        pieces = [(cbo[:, oi * 128:oi * 128 + 128], ones_row[:, :])]
                        pieces += [(wt[:, kt, :], oT[kt][:, :]) for kt in range(NKT)]
                        mm_group(ps[:, :], pieces)
                        ut = uf_tile(f"u1_{oi}")
                        nc.vector.scalar_tensor_tensor(
                            ut[:, :], res_ap(zres[b][oi]),
                            sprev[:, oi:oi + 1], ps[:, :],
                            op0=AL.mult, op1=AL.add)
                        u1.append(ut)
                    scope.__exit__(None, None, None)
                    with nc.named_scope(f"L{l}b{b}_ln1"):
                        z1[b] = layer_norm(u1, f"z1_{l}_{b}")
                # ---- FFN
                z2mm = [None] * BC
                z2res = [None] * BC
                for b in range(BC):
                    z1b = z1[b]
                    scope = nc.named_scope(f"L{l}b{b}_ffn")
                    scope.__enter__()
                    h1 = []
                    for oi in range(NFT):
                        wt = wt_tile(f"w1_{oi}")
                        nc.sync.dma_start(wt[:, :, :], io[f"w1{l}"][oi])
                        ps = ptile("ps_f1")
                        mm_group(ps[:, :],
                                 [(wt[:, kt, :], z1b[kt][:, :]) for kt in range(NKT)])
                        ht = big(f"h1_{oi}")
                        nc.scalar.activation(ht[:, :], ps[:, :], AF.Relu,
                                             bias=lcn[("b1c", l)][:, oi:oi + 1])
                        h1.append(ht)
                    u2 = []
                    for oi in range(NKT):
                        wta = wt_tile(f"w2a_{oi}")
                        nc.sync.dma_start(wta[:, :, :], io[f"w2{l}"][oi, :, 0:8, :])
                        wtb = wt_tile(f"w2b_{oi}")
                        nc.sync.dma_start(wtb[:, :, :], io[f"w2{l}"][oi, :, 8:16, :])
                        ps = ptile("ps_f2")
                        pieces = [(cb2[:, oi * 128:oi * 128 + 128], ones_row[:, :])]
                        pieces += [(wta[:, kt, :], h1[kt][:, :]) for kt in range(8)]
                        pieces += [(wtb[:, kt, :], h1[8 + kt][:, :]) for kt in range(8)]
                        mm_group(ps[:, :], pieces)
                        ut = uf_tile(f"u2_{oi}")
                        nc.vector.scalar_tensor_tensor(
                            ut[:, :], z1b[oi][:, :],
                            s1[:, oi:oi + 1], ps[:, :],
                            op0=AL.mult, op1=AL.add)
                        u2.append(ut)
                    scope.__exit__(None, None, None)
                    last = (l == L - 1)
                    with nc.named_scope(f"L{l}b{b}_ln2"):
                        zt = layer_norm(u2, f"z2_{l}_{b}",
                                        out_dt=(f32r if last else bf16))
                    z2res[b] = zt
                    z2mm[b] = zt
                    if last:
                        # mean-pool this batch row block now so the DVE work
                        # overlaps the other block's FFN
                        for kt in range(NKT):
                            repz = ap.tile([128, 1], f32, name="repz",
                                           tag="sm", bufs=24)
                            nc.vector.tensor_reduce(
                                repz[:, :], zt[kt][:, :].bitcast(f32),
                                axis=AX.X, op=AL.add)
                            repl = ap.tile([128, 1], f32, name="repl",
                                           tag="sm", bufs=24)
                            nc.vector.tensor_scalar(
                                repl[:, :], repz[:, :],
                                slmul_sb[:, kt:kt + 1], blast_sb[:, kt:kt + 1],
                                op0=AL.mult, op1=AL.add)
                            nc.sync.dma_start(
                                agin[kt * 128:kt * 128 + 128, b:b + 1],
                                repl[:, :])
                zres, zmm = z2res, z2mm

            # ================= AllGather rep =================
            tail_scope = nc.named_scope("tail_route")
            tail_scope.__enter__()
            nc.gpsimd.collective_compute(
                "AllGather", AL.bypass, replica_groups=RG,
                ins=[agin[:, :]], outs=[agout[:, :]])
            repT = []
            for kt in range(NKT):
                rt = ap.tile([128, 16], f32, name=f"repT_{kt}", tag="sm", bufs=24)
                nc.sync.dma_start(
                    rt[:, :].rearrange("p (c j) -> p c j", c=NC),
                    agout[:, :].rearrange("(c dt p) j -> dt p c j",
                                          c=NC, dt=NKT)[kt])
                repT.append(rt)

            # ================= gating (fp32) =================
            ps_g = ptile("ps_g")
            pieces = [(ones16f[:, :], bgr_sb[:, :])]
            pieces += [(repT[kt][:, :], wg_sb[:, kt, :]) for kt in range(NKT)]
            mm_group(ps_g[0:16, 0:E], pieces)
            glog = ap.tile([16, E], f32, name="glog", tag="sm", bufs=24)
            nc.vector.tensor_copy(glog[:, :], ps_g[0:16, 0:E])
            negmax = ap.tile([16, 1], f32, name="negmax", tag="sm", bufs=24)
            nc.vector.tensor_reduce(negmax[:, :], glog[:, :], axis=AX.X,
                                    op=AL.max, negate=True)
            gexp = ap.tile([16, E], f32, name="gexp", tag="sm", bufs=24)
            sumexp = ap.tile([16, 1], f32, name="sumexp", tag="sm", bufs=24)
            nc.scalar.activation(gexp[:, :], glog[:, :], AF.Exp,
                                 bias=negmax[:, :], scale=1.0,
                                 accum_out=sumexp[:, :])
            grec = ap.tile([16, 1], f32, name="grec", tag="sm", bufs=24)
            nc.vector.reciprocal(grec[:, :], sumexp[:, :])
            gw = ap.tile([16, E], f32, name="gw", tag="sm", bufs=24)
            nc.vector.tensor_scalar_mul(gw[:, :], gexp[:, :], grec[:, :])
            nc.sync.dma_start(gw_out[:, :], gw[:, :])
            gmax8 = ap.tile([16, 8], f32, name="gmax8", tag="sm", bufs=24)
            gidx8 = ap.tile([16, 8], u32, name="gidx8", tag="sm", bufs=24)
            nc.vector.max_with_indices(gmax8[:, :], gidx8[:, :], gw[:, :])
            gidx8b = ap.tile([16, 8], u32, name="gidx8b", tag="sm", bufs=24)
            gmax8b = ap.tile([16, 8], f32, name="gmax8b", tag="sm", bufs=24)
            nc.vector.max_with_indices(gmax8b[:, :], gidx8b[:, :], glog[:, :])
            idx32 = ap.tile([16, 1], i32, name="idx32", tag="sm", bufs=24)
            nc.vector.tensor_copy(idx32[:, :], gidx8[:, 0:1])
            nc.sync.dma_start(idx_out[:, :], idx32[:, :])

            # expert mask row (1,16) via DRAM round-trip transpose
            idxf = ap.tile([16, 1], f32, name="idxf", tag="sm", bufs=24)
            nc.vector.tensor_copy(idxf[:, :], gidx8b[:, 0:1])
            drm = dr.tile([16, 1], f32, name="drm")
            nc.sync.dma_start(drm[:, :], idxf[:, :])
            idxrow = ap.tile([1, 16], f32, name="idxrow", tag="sm", bufs=24)
            nc.sync.dma_start(idxrow[:, :], drm[:, :].rearrange("b one -> one b"))
            maskrow = ap.tile([1, 16], f32, name="maskrow", tag="sm", bufs=24)
            nc.vector.tensor_scalar(maskrow[:, :], idxrow[:, :], ecmp_sb[:, :],
                                    None, op0=AL.is_equal)
            maskb = ap.tile([128, 16], f32, name="maskb", tag="sm", bufs=24)
            nc.gpsimd.partition_broadcast(maskb[:, :], maskrow[:, :])

            # bf16 copies of repT for the expert matmul
            repB = []
            for kt in range(NKT):
                rb_ = ap.tile([128, 16], bf16, name=f"repB_{kt}", tag="sm", bufs=24)
                nc.vector.tensor_copy(rb_[:, :], repT[kt][:, :])
                repB.append(rb_)
            ber_b = ap.tile([1, D], bf16, name="ber_b", tag="rowc", bufs=3)
            nc.vector.tensor_copy(ber_b[:, :], ber_sb[:, :])
            # ================= expert matmul (bf16) + AllReduce =================
            arin = dr.tile([D, 16], bf16, name="arin")
            arout = dr.tile([D, 16], bf16, name="arout", addr_space="Shared")
            for oi in range(NKT):
                we = wp.tile([128, NKT, 128], bf16, name=f"we_{oi}", tag="wt",
                             bufs=BUFS_WT)
                nc.sync.dma_start(we[:, :, :], io["weT"][oi])
                ps = ptile("ps_e")
                pieces = [(ber_b[:, oi * 128:oi * 128 + 128], ones16b[:, :])]
                pieces += [(we[:, kt, :], repB[kt][:, :]) for kt in range(NKT)]
                mm_group(ps[0:128, 0:16], pieces)
                contrib = ap.tile([128, 16], bf16, name="contrib", tag="sm", bufs=24)
                nc.vector.tensor_tensor(contrib[:, :], ps[0:128, 0:16],
                                        maskb[:, :], op=AL.mult)
                nc.sync.dma_start(arin[oi * 128:oi * 128 + 128, :], contrib[:, :])
            nc.gpsimd.collective_compute(
                "AllReduce", AL.add, replica_groups=RG,
                ins=[arin[:, :]], outs=[arout[:, :]])
            eo = []
            for kt in range(NKT):
                er = ap.tile([128, 16], bf16, name=f"eor_{kt}", tag="sm", bufs=24)
                nc.sync.dma_start(er[:, :], arout[kt * 128:kt * 128 + 128, :])
                eo.append(er)

            tail_scope.__exit__(None, None, None)
            # ================= head (vocab shard) =================
            head_scope = nc.named_scope("head")
            head_scope.__enter__()
            for vi in range(13):
                wh = wp.tile([128, NKT, 512], bf16, name=f"wh_{vi}",
                             tag="wvh", bufs=BUFS_WVH)
                nc.sync.dma_start(wh[:, :, :], io["whT"][vi])
                bhc = ap.tile([1, 512], bf16, name="bhc", tag="bh", bufs=2)
                nc.sync.dma_start(bhc[:, :], io["bhr"][:, vi * 512:vi * 512 + 512])
                ps = ptile("ps_h")
                pieces = [(ones16b[:, :], bhc[:, :])]
                pieces += [(eo[kt][:, :], wh[:, kt, :]) for kt in range(NKT)]
                mm_group(ps[0:16, :], pieces)
                lg = ap.tile([16, 512], f32, name="lg", tag="lg", bufs=2)
                nc.vector.tensor_copy(lg[:, :], ps[0:16, :])
                nc.sync.dma_start(logits_s[:, vi * 512:vi * 512 + 512], lg[:, :])
            head_scope.__exit__(None, None, None)

    nc.compile()
    return nc


_NC_CACHE = None


def _get_nc():
    global _NC_CACHE
    if _NC_CACHE is None:
        _NC_CACHE = _build_nc()
    return _NC_CACHE


def _prep_inputs(inputs):
    g = {k: np.asarray(v, dtype=np.float32) for k, v in inputs.items()}
    x = g["x"]

    def tile4(w, no, nk, pk, po, dt=ml_dtypes.bfloat16):
        # w: (nk*pk, no*po) -> [no, pk, nk, po]: each [no] slice DMAs with
        # fully-contiguous per-partition rows
        return np.ascontiguousarray(
            w.reshape(nk, pk, no, po).transpose(2, 1, 0, 3).astype(dt))

    bfl = ml_dtypes.bfloat16
    com = {}
    for l in range(L):
        sprev = np.ones(D, np.float32) if l == 0 else g["ln2_s"][l - 1]
        bprev = np.zeros(D, np.float32) if l == 0 else g["ln2_b"][l - 1]
        Wqkv = g["Wqkv"][l]                      # (3D, D)
        beff = g["bqkv"][l] + Wqkv @ bprev       # (3D,)
        Weff = (Wqkv * sprev[None, :]).copy()
        Weff[:D] *= SCALE
        beff = beff.copy()
        beff[:D] *= SCALE
        WqkvT = np.ascontiguousarray(Weff.T)     # (D, 3D)
        com[f"wqkv{l}"] = tile4(WqkvT[:, :2 * D], 16, NKT, 128, 128)
        com[f"wv{l}"] = tile4(WqkvT[:, 2 * D:], 2, NKT, 128, 512)
        com[f"bqkc{l}"] = np.ascontiguousarray(beff[:2 * D].reshape(2 * D, 1))
        com[f"bvr{l}"] = np.ascontiguousarray(
            beff[2 * D:].reshape(1, D).astype(bfl))
        com[f"wo{l}"] = tile4(np.ascontiguousarray(g["Wo"][l].T), NKT, NKT, 128, 128)
        com[f"cbo{l}"] = np.ascontiguousarray(
            (g["bo"][l] + bprev).reshape(1, D).astype(bfl))
        s1 = g["ln1_s"][l]
        b1ln = g["ln1_b"][l]
        W1 = g["W1"][l]                          # (DFF, D)
        com[f"w1{l}"] = tile4(np.ascontiguousarray((W1 * s1[None, :]).T),
                              NFT, NKT, 128, 128)
        com[f"b1c{l}"] = np.ascontiguousarray((g["b1"][l] + W1 @ b1ln).reshape(DFF, 1))
        com[f"w2{l}"] = tile4(np.ascontiguousarray(g["W2"][l].T), NKT, NFT, 128, 128)
        com[f"cb2{l}"] = np.ascontiguousarray(
            (g["b2"][l] + b1ln).reshape(1, D).astype(bfl))
        com[f"sprev{l}"] = sprev.reshape(D, 1).copy()
        com[f"s1_{l}"] = s1.reshape(D, 1).copy()
    com["slmul"] = (g["ln2_s"][L - 1] / S).reshape(D, 1).copy()
    com["blast"] = g["ln2_b"][L - 1].reshape(D, 1).copy()
    com["wg"] = np.ascontiguousarray(g["Wg"].T)      # (D, E)
    com["bgr"] = g["bg"].reshape(1, E).copy()
    com["ones_col"] = np.ones((128, 1), np.float32)
    com["ones_row"] = np.ones((1, 512), bfl)
    com["ones16f"] = np.ones((1, 16), np.float32)

    WhT_pad = np.zeros((D, VPAD), np.float32)
    WhT_pad[:, :V] = g["Wh"].T
    bh_pad = np.zeros(VPAD, np.float32)
    bh_pad[:V] = g["bh"]

    in_maps = []
    for c in range(NC):
        m = dict(com)
        xs = x[:, BC * c:BC * (c + 1), :]            # (S, BC, D)
        xt_ = np.ascontiguousarray(
            xs.transpose(2, 1, 0).reshape(NKT, 128, T))
        m["xT"] = xt_
        m["xTb"] = xt_.astype(bfl)
        m["weT"] = tile4(np.ascontiguousarray(g["We"][c].T), NKT, NKT, 128, 128)
        m["ber"] = g["be"][c].reshape(1, D).copy()
        m["ecmp"] = np.full((1, 1), float(c), np.float32)
        whc = WhT_pad[:, c * VS:(c + 1) * VS]
        m["whT"] = tile4(whc, 13, NKT, 128, 512)
        m["bhr"] = np.ascontiguousarray(
            bh_pad[c * VS:(c + 1) * VS].reshape(1, VS).astype(bfl))
        in_maps.append(m)
    return in_maps


LAST_RESULTS = None


def kernel(**inputs):
    global LAST_RESULTS
    in_maps = _prep_inputs(inputs)
    nc = _get_nc()
    res = run_bass_kernel_spmd(
        nc, in_maps, core_ids=list(range(NC)),
        trace=os.environ.get("KERNEL_TRACE") == "1")
    LAST_RESULTS = res
    logits = np.concatenate([res.results[c]["logits_s"] for c in range(NC)],
                            axis=1)[:, :V]
    gating = res.results[0]["gw_out"]
    idx = res.results[0]["idx_out"].ravel().astype(np.int32)
    return np.ascontiguousarray(logits), np.ascontiguousarray(gating), idx


# revision 26
# speedup vs baseline: 1.0060x; 1.0060x over previous
"""MoE transformer encoder kernel for 8 TRN2 NeuronCores.

Sharding: data-parallel over batch (B=16 -> 2 per core) for the 4 encoder
layers; expert-parallel experts (1 per core, masked-sum AllReduce); head
matmul sharded over vocab columns. Small AllGather of pooled reps.

Layout: activations are feature-major xT[D, tokens] on-chip; weights are
host-pre-transposed and pre-tiled for contiguous DMA. Projection matmuls
run in bf16 (full PE stream rate; fp32/f32r stream at half rate); the
residual stream / LayerNorm stats run in f32r, the gating/expert path in
fp32, and the final layer's LN output stays f32r so the pooled reps that
feed the router are ~1e-4 accurate. LN stats use ones-matmul partition
reductions; LN scale/bias are folded into adjacent weights on host.
"""
import os
import sys

for _p in ("/opt/trn_rl_repo", "/root/.axon_site/_ro/trn_rl_repo"):
    if _p not in sys.path:
        sys.path.append(_p)

import ml_dtypes
import numpy as np
import concourse.bacc as bacc
import concourse.mybir as mybir
from concourse.tile import TileContext
from concourse.bass_utils import run_bass_kernel_spmd

S, B, D, H, L = 512, 16, 1024, 16, 4
DFF, E, V = 2048, 8, 50257
HD = D // H
SCALE = float(1.0 / np.sqrt(HD))
EPS = 1e-5
NC = 8
BC = B // NC          # 2 batch rows per core
T = S * BC            # 1024 tokens per core
VS = 6656             # 13*512 vocab cols per core
VPAD = VS * NC
NKT = D // 128        # 8 contraction tiles over D
NFT = DFF // 128      # 16 tiles over DFF

f32 = mybir.dt.float32
f32r = mybir.dt.float32r
bf16 = mybir.dt.bfloat16
i32 = mybir.dt.int32
u32 = mybir.dt.uint32
AL = mybir.AluOpType
AF = mybir.ActivationFunctionType
AX = mybir.AxisListType

RG = [list(range(NC))]

# SBUF slot budgets (bufs per tag)
BUFS_Z = 14      # bf16 residual-stream tiles (1KB/part each)
BUFS_ZF = 9      # f32r final-layer residual tiles (2KB)
BUFS_M5 = 42     # bf16 transient activations (1KB)
BUFS_UF = 16     # f32r pre-norm residual + squares (2KB)
BUFS_ST = 5
BUFS_WT = 5      # bf16 weight tiles (2KB)
BUFS_WVH = 6     # bf16 (128,8,512) rhs weight blocks (8KB)


def _build_nc():
    nc = bacc.Bacc(num_devices=NC)

    def inp(name, shape, dt=bf16):
        return nc.dram_tensor(name, shape, dt, kind="ExternalInput")

    io = {}
    io["xT"] = inp("xT", (NKT, 128, T), f32r)
    io["xTb"] = inp("xTb", (NKT, 128, T))
    for l in range(L):
        io[f"wqkv{l}"] = inp(f"wqkv{l}", (16, 128, NKT, 128))   # [oi, p, kt, o]
        io[f"wv{l}"] = inp(f"wv{l}", (2, 128, NKT, 512))        # [oc, p, kt, o]
        io[f"bqkc{l}"] = inp(f"bqkc{l}", (2 * D, 1), f32)
        io[f"bvr{l}"] = inp(f"bvr{l}", (1, D))
        io[f"wo{l}"] = inp(f"wo{l}", (NKT, 128, NKT, 128))
        io[f"cbo{l}"] = inp(f"cbo{l}", (1, D))
        io[f"w1{l}"] = inp(f"w1{l}", (NFT, 128, NKT, 128))
        io[f"b1c{l}"] = inp(f"b1c{l}", (DFF, 1), f32)
        io[f"w2{l}"] = inp(f"w2{l}", (NKT, 128, NFT, 128))
        io[f"cb2{l}"] = inp(f"cb2{l}", (1, D))
        io[f"sprev{l}"] = inp(f"sprev{l}", (D, 1), f32)
        io[f"s1_{l}"] = inp(f"s1_{l}", (D, 1), f32)
    io["slmul"] = inp("slmul", (D, 1), f32)
    io["blast"] = inp("blast", (D, 1), f32)
    io["wg"] = inp("wg", (D, E), f32)
    io["bgr"] = inp("bgr", (1, E), f32)
    io["weT"] = inp("weT", (NKT, 128, NKT, 128))
    io["ber"] = inp("ber", (1, D), f32)
    io["ecmp"] = inp("ecmp", (1, 1), f32)
    io["whT"] = inp("whT", (13, 128, NKT, 512))
    io["bhr"] = inp("bhr", (1, VS))
    io["ones_col"] = inp("ones_col", (128, 1), f32r)
    io["ones_row"] = inp("ones_row", (1, 512))
    io["ones16f"] = inp("ones16f", (1, 16), f32)

    logits_s = nc.dram_tensor("logits_s", (16, VS), f32, kind="ExternalOutput")
    gw_out = nc.dram_tensor("gw_out", (16, E), f32, kind="ExternalOutput")
    idx_out = nc.dram_tensor("idx_out", (16, 1), i32, kind="ExternalOutput")

    with TileContext(nc) as tc:
        with tc.tile_pool(name="cn", bufs=1) as cn, \
             tc.tile_pool(name="wp", bufs=1) as wp, \
             tc.tile_pool(name="ap", bufs=1) as ap, \
             tc.tile_pool(name="dr", bufs=1, space="DRAM") as dr, \
             tc.tile_pool(name="pp", bufs=1, space="PSUM") as pp:

            def ptile(name):
                return pp.tile([128, 512], f32, name=name, tag="pb", bufs=8)

            def mm_group(psum_ap, pieces):
                n = len(pieces)
                for i, (lh, rh) in enumerate(pieces):
                    nc.tensor.matmul(psum_ap, lh, rh,
                                     start=(i == 0), stop=(i == n - 1))

            def ztile(name, dt=bf16):
                if dt is f32r:
                    return ap.tile([128, 512], f32r, name=name, tag="zf",
                                   bufs=BUFS_ZF)
                return ap.tile([128, 512], bf16, name=name, tag="z", bufs=BUFS_Z)

            def big(name):     # bf16 transient activation slot
                return ap.tile([128, 512], bf16, name=name, tag="m5", bufs=BUFS_M5)

            def uf_tile(name):  # f32r pre-norm residual / squares
                return ap.tile([128, 512], f32r, name=name, tag="uf", bufs=BUFS_UF)

            def stt_(name):
                return ap.tile([1, 512], f32, name=name, tag="st", bufs=BUFS_ST)

            def wt_tile(name):
                return wp.tile([128, 8, 128], bf16, name=name, tag="wt",
                               bufs=BUFS_WT)

            # layer-0 input: f32r residual copy + host-cast bf16 matmul copy
            zres = [[None] * NKT for _ in range(BC)]
            zmm = [[None] * NKT for _ in range(BC)]
            for b in range(BC):
                for kt in range(NKT):
                    zb = ztile(f"zinb_{b}_{kt}")
                    nc.sync.dma_start(zb[:, :],
                                      io["xTb"][kt, :, b * 512:(b + 1) * 512])
                    zr = ztile(f"zin_{b}_{kt}", f32r)
                    nc.sync.dma_start(zr[:, :],
                                      io["xT"][kt, :, b * 512:(b + 1) * 512])
                    zres[b][kt] = zr
                    zmm[b][kt] = zb

            # ---------------- constants into SBUF
            ones_col = cn.tile([128, 1], f32r, name="ones_col_sb")
            nc.sync.dma_start(ones_col[:, :], io["ones_col"][:, :])
            ones_row = cn.tile([1, 512], bf16, name="ones_row_sb")
            nc.sync.dma_start(ones_row[:, :], io["ones_row"][:, :])
            ones16f = cn.tile([1, 16], f32, name="ones16f_sb")
            nc.sync.dma_start(ones16f[:, :], io["ones16f"][:, :])
            ecmp_sb = cn.tile([1, 1], f32, name="ecmp_sb")
            nc.sync.dma_start(ecmp_sb[:, :], io["ecmp"][:, :])
            ones_col_bf = cn.tile([128, 1], bf16, name="ones_col_bf")
            nc.vector.tensor_copy(ones_col_bf[:, :], ones_col[:, :].bitcast(f32))
            ones16b = cn.tile([1, 16], bf16, name="ones16b")
            nc.vector.tensor_copy(ones16b[:, :], ones16f[:, :])

            lcn = {}
            for l in range(L):
                for nm, src, w in (("bqkc", f"bqkc{l}", 16), ("b1c", f"b1c{l}", 16),
                                   ("sprev", f"sprev{l}", 8), ("s1", f"s1_{l}", 8)):
                    t = cn.tile([128, w], f32, name=f"{nm}{l}_sb")
                    nc.sync.dma_start(
                        t[:, :], io[src][:, :].rearrange("(o p) one -> p (o one)", p=128))
                    lcn[(nm, l)] = t

            def row_const(src_ap, name):
                t = ap.tile([1, D], bf16, name=name, tag="rowc", bufs=3)
                nc.sync.dma_start(t[:, :], src_ap)
                return t

            slmul_sb = cn.tile([128, 8], f32, name="slmul_sb")
            nc.sync.dma_start(slmul_sb[:, :],
                              io["slmul"][:, :].rearrange("(o p) one -> p (o one)", p=128))
            blast_sb = cn.tile([128, 8], f32, name="blast_sb")
            nc.sync.dma_start(blast_sb[:, :],
                              io["blast"][:, :].rearrange("(o p) one -> p (o one)", p=128))
            wg_sb = cn.tile([128, NKT, E], f32, name="wg_sb")
            nc.sync.dma_start(wg_sb[:, :, :],
                              io["wg"][:, :].rearrange("(kt p) e -> p kt e", p=128))
            bgr_sb = cn.tile([1, E], f32, name="bgr_sb")
            nc.sync.dma_start(bgr_sb[:, :], io["bgr"][:, :])
            ber_sb = cn.tile([1, D], f32, name="ber_sb")
            nc.sync.dma_start(ber_sb[:, :], io["ber"][:, :])

            def res_ap(t):
                return t[:, :].bitcast(f32) if t.dtype == f32r else t[:, :]

            def layer_norm(u_tiles, zname, out_dt=bf16):
                """u_tiles: 8 x (128,512) f32r -> 8 normalized z tiles."""
                usq = []
                for kt in range(NKT):
                    sq = big(f"usq_{kt}")
                    if kt % 2 == 0:
                        nc.scalar.activation(sq[:, :], u_tiles[kt][:, :], AF.Square)
                    else:
                        nc.vector.tensor_tensor(
                            sq[:, :], u_tiles[kt][:, :].bitcast(f32),
                            u_tiles[kt][:, :].bitcast(f32), op=AL.mult)
                    usq.append(sq)
                ps_m = ptile("ps_m")
                mm_group(ps_m[0:1, :],
                         [(ones_col[:, :], u_tiles[kt][:, :]) for kt in range(NKT)])
                ps_q = ptile("ps_q")
                mm_group(ps_q[0:1, :],
                         [(ones_col_bf[:, :], usq[kt][:, :]) for kt in range(NKT)])
                mcol = stt_("mcol")
                nc.scalar.mul(mcol[:, :], ps_m[0:1, :], 1.0 / D)
                qcol = stt_("qcol")
                nc.scalar.mul(qcol[:, :], ps_q[0:1, :], 1.0 / D)
                msq = stt_("msq")
                nc.vector.tensor_tensor(msq[:, :], mcol[:, :], mcol[:, :], op=AL.mult)
                var = stt_("var")
                nc.vector.tensor_tensor(var[:, :], qcol[:, :], msq[:, :],
                                        op=AL.subtract)
                vpe = stt_("vpe")
                nc.vector.tensor_single_scalar(vpe[:, :], var[:, :], EPS, op=AL.add)
                rcp = stt_("rcp")
                nc.vector.reciprocal(rcp[:, :], vpe[:, :])
                rstd = stt_("rstd")
                nc.scalar.sqrt(rstd[:, :], rcp[:, :])
                mb = ap.tile([128, 512], f32, name="mb", tag="bc", bufs=2)
                nc.gpsimd.partition_broadcast(mb[:, :], mcol[:, :])
                rb = ap.tile([128, 512], f32, name="rb", tag="bc", bufs=2)
                nc.gpsimd.partition_broadcast(rb[:, :], rstd[:, :])
                zt = []
                for kt in range(NKT):
                    t1 = uf_tile(f"lnt_{kt}")
                    eng = nc.gpsimd if kt % 2 == 0 else nc.vector
                    eng.tensor_tensor(
                        t1[:, :], u_tiles[kt][:, :].bitcast(f32), mb[:, :],
                        op=AL.subtract)
                    zo = ztile(f"{zname}_{kt}", out_dt)
                    eng2 = nc.gpsimd if kt % 2 == 1 else nc.vector
                    eng2.tensor_tensor(
                        zo[:, :], t1[:, :].bitcast(f32), rb[:, :], op=AL.mult)
                    zt.append(zo)
                return zt

            # DRAM bounce tensors for the routing collectives
            agin = dr.tile([D, BC], f32, name="agin")
            agout = dr.tile([NC * D, BC], f32, name="agout", addr_space="Shared")

            # ================= encoder layers =================
            for l in range(L):
                sprev = lcn[("sprev", l)]
                s1 = lcn[("s1", l)]
                bvr = row_const(io[f"bvr{l}"][:, :], f"bvr{l}_sb")
                cbo = row_const(io[f"cbo{l}"][:, :], f"cbo{l}_sb")
                cb2 = row_const(io[f"cb2{l}"][:, :], f"cb2{l}_sb")
                z1 = [None] * BC
                for b in range(BC):
                    zb = zmm[b]
                    scope = nc.named_scope(f"L{l}b{b}_attn")
                    scope.__enter__()
                    # ---- q,k projections (feature-major)
                    qk = []
                    for oi in range(16):
                        wt = wt_tile(f"wqkv_{oi}")
                        nc.sync.dma_start(wt[:, :, :], io[f"wqkv{l}"][oi])
                        ps = ptile("ps_qk")
                        mm_group(ps[:, :],
                                 [(wt[:, kt, :], zb[kt][:, :]) for kt in range(NKT)])
                        qt = big(f"qk_{oi}")
                        nc.scalar.activation(qt[:, :], ps[:, :], AF.Identity,
                                             bias=lcn[("bqkc", l)][:, oi:oi + 1])
                        qk.append(qt)
                    qts, kts = qk[:8], qk[8:]
                    # ---- v (token-major)
                    vts = [[None, None] for _ in range(4)]
                    for oc in range(2):
                        wv = wp.tile([128, NKT, 512], bf16, name=f"wv_{oc}",
                                     tag="wvh", bufs=BUFS_WVH)
                        nc.sync.dma_start(wv[:, :, :], io[f"wv{l}"][oc])
                        for ti in range(4):
                            ps = ptile("ps_v")
                            pieces = [(ones_row[:, 0:128],
                                       bvr[:, oc * 512:oc * 512 + 512])]
                            pieces += [(zb[kt][:, ti * 128:ti * 128 + 128],
                                        wv[:, kt, :]) for kt in range(NKT)]
                            mm_group(ps[:, :], pieces)
                            # 65-wide head blocks: col 64 = ones so the AV
                            # matmul emits the softmax denominator in row 64
                            vt = ap.tile([128, 8, 65], bf16, name=f"v_{ti}_{oc}",
                                         tag="m5", bufs=BUFS_M5)
                            nc.scalar.copy(
                                vt[:, :, 0:64],
                                ps[:, :].rearrange("p (h o) -> p h o", h=8))
                            nc.vector.memset(vt[:, :, 64:65], 1.0)
                            vts[ti][oc] = vt
                    # ---- attention per head (interleaved accumulation keeps
                    # per-head PSUM footprint at ~4 banks -> heads overlap)
                    oT = [big(f"oT_{fi}") for fi in range(NKT)]
                    for h in range(H):
                        fi, ro = h // 2, (h % 2) * 64
                        qh = qts[fi][ro:ro + 64, :]
                        kh = kts[fi][ro:ro + 64, :]
                        oc, hh = h // 8, h % 8
                        ps_av = ptile("ps_av")
                        for kt2 in range(4):
                            ps_s = ptile("ps_s")
                            nc.tensor.matmul(ps_s[:, :],
                                             kh[:, kt2 * 128:kt2 * 128 + 128],
                                             qh[:, :], start=True, stop=True)
                            ex = big(f"exp_{kt2}")
                            nc.scalar.activation(ex[:, :], ps_s[:, :], AF.Exp)
                            nc.tensor.matmul(ps_av[0:65, :],
                                             vts[kt2][oc][:, hh, :],
                                             ex[:, :], start=(kt2 == 0),
                                             stop=(kt2 == 3))
                        rec = stt_("rec")
                        nc.vector.reciprocal(rec[:, :], ps_av[64:65, :])
                        recb = ap.tile([64, 512], f32, name="recb", tag="bc2", bufs=2)
                        nc.gpsimd.partition_broadcast(recb[:, :], rec[:, :])
                        nc.vector.tensor_tensor(oT[fi][ro:ro + 64, :],
                                                ps_av[0:64, :], recb[:, :],
                                                op=AL.mult)
                    # ---- Wo projection + residual -> u1 -> LN1
                    u1 = []
                    for oi in range(NKT):
                        wt = wt_tile(f"wo_{oi}")
                        nc.sync.dma_start(wt[:, :, :], io[f"wo{l}"][oi])
                        ps = ptile("ps_wo")
                        pieces = [(cbo[:, oi * 128:oi * 128 + 128], ones_row[:, :])]
                        pieces += [(wt[:, kt, :], oT[kt][:, :]) for kt in range(NKT)]
                        mm_group(ps[:, :], pieces)
                        ut = uf_tile(f"u1_{oi}")
                        nc.vector.scalar_tensor_tensor(
                            ut[:, :], res_ap(zres[b][oi]),
                            sprev[:, oi:oi + 1], ps[:, :],
                            op0=AL.mult, op1=AL.add)
                        u1.append(ut)
                    scope.__exit__(None, None, None)
                    with nc.named_scope(f"L{l}b{b}_ln1"):
                        z1[b] = layer_norm(u1, f"z1_{l}_{b}")
                # ---- FFN
                z2mm = [None] * BC
                z2res = [None] * BC
                for b in range(BC):
                    z1b = z1[b]
                    scope = nc.named_scope(f"L{l}b{b}_ffn")
                    scope.__enter__()
                    h1 = []
                    for oi in range(NFT):
                        wt = wt_tile(f"w1_{oi}")
                        nc.sync.dma_start(wt[:, :, :], io[f"w1{l}"][oi])
                        ps = ptile("ps_f1")
                        mm_group(ps[:, :],
                                 [(wt[:, kt, :], z1b[kt][:, :]) for kt in range(NKT)])
                        ht = big(f"h1_{oi}")
                        nc.scalar.activation(ht[:, :], ps[:, :], AF.Relu,
                                             bias=lcn[("b1c", l)][:, oi:oi + 1])
                        h1.append(ht)
                    u2 = []
                    for oi in range(NKT):
                        wta = wt_tile(f"w2a_{oi}")
                        nc.sync.dma_start(wta[:, :, :], io[f"w2{l}"][oi, :, 0:8, :])
                        wtb = wt_tile(f"w2b_{oi}")
                        nc.sync.dma_start(wtb[:, :, :], io[f"w2{l}"][oi, :, 8:16, :])
                        ps = ptile("ps_f2")
                        pieces = [(cb2[:, oi * 128:oi * 128 + 128], ones_row[:, :])]
                        pieces += [(wta[:, kt, :], h1[kt][:, :]) for kt in range(8)]
                        pieces += [(wtb[:, kt, :], h1[8 + kt][:, :]) for kt in range(8)]
                        mm_group(ps[:, :], pieces)
                        ut = uf_tile(f"u2_{oi}")
                        nc.vector.scalar_tensor_tensor(
                            ut[:, :], z1b[oi][:, :],
                            s1[:, oi:oi + 1], ps[:, :],
                            op0=AL.mult, op1=AL.add)
                        u2.append(ut)
                    scope.__exit__(None, None, None)
                    last = (l == L - 1)
                    with nc.named_scope(f"L{l}b{b}_ln2"):
                        zt = layer_norm(u2, f"z2_{l}_{b}",
                                        out_dt=(f32r if last else bf16))
                    z2res[b] = zt
                    z2mm[b] = zt
                    if last:
                        # mean-pool this batch row block now so the DVE work
                        # overlaps the other block's FFN
                        for kt in range(NKT):
                            repz = ap.tile([128, 1], f32, name="repz",
                                           tag="sm", bufs=24)
                            nc.vector.tensor_reduce(
                                repz[:, :], zt[kt][:, :].bitcast(f32),
                                axis=AX.X, op=AL.add)
                            repl = ap.tile([128, 1], f32, name="repl",
                                           tag="sm", bufs=24)
                            nc.vector.tensor_scalar(
                                repl[:, :], repz[:, :],
                                slmul_sb[:, kt:kt + 1], blast_sb[:, kt:kt + 1],
                                op0=AL.mult, op1=AL.add)
                            nc.sync.dma_start(
                                agin[kt * 128:kt * 128 + 128, b:b + 1],
                                repl[:, :])
                zres, zmm = z2res, z2mm

            # ================= AllGather rep =================
            tail_scope = nc.named_scope("tail_route")
            tail_scope.__enter__()
            nc.gpsimd.collective_compute(
                "AllGather", AL.bypass, replica_groups=RG,
                ins=[agin[:, :]], outs=[agout[:, :]])
            repT = []
            for kt in range(NKT):
                rt = ap.tile([128, 16], f32, name=f"repT_{kt}", tag="sm", bufs=24)
                nc.sync.dma_start(
                    rt[:, :].rearrange("p (c j) -> p c j", c=NC),
                    agout[:, :].rearrange("(c dt p) j -> dt p c j",
                                          c=NC, dt=NKT)[kt])
                repT.append(rt)

            # ================= gating (fp32) =================
            ps_g = ptile("ps_g")
            pieces = [(ones16f[:, :], bgr_sb[:, :])]
            pieces += [(repT[kt][:, :], wg_sb[:, kt, :]) for kt in range(NKT)]
            mm_group(ps_g[0:16, 0:E], pieces)
            glog = ap.tile([16, E], f32, name="glog", tag="sm", bufs=24)
            nc.vector.tensor_copy(glog[:, :], ps_g[0:16, 0:E])
            negmax = ap.tile([16, 1], f32, name="negmax", tag="sm", bufs=24)
            nc.vector.tensor_reduce(negmax[:, :], glog[:, :], axis=AX.X,
                                    op=AL.max, negate=True)
            gexp = ap.tile([16, E], f32, name="gexp", tag="sm", bufs=24)
            sumexp = ap.tile([16, 1], f32, name="sumexp", tag="sm", bufs=24)
            nc.scalar.activation(gexp[:, :], glog[:, :], AF.Exp,
                                 bias=negmax[:, :], scale=1.0,
                                 accum_out=sumexp[:, :])
            grec = ap.tile([16, 1], f32, name="grec", tag="sm", bufs=24)
            nc.vector.reciprocal(grec[:, :], sumexp[:, :])
            gw = ap.tile([16, E], f32, name="gw", tag="sm", bufs=24)
            nc.vector.tensor_scalar_mul(gw[:, :], gexp[:, :], grec[:, :])
            nc.sync.dma_start(gw_out[:, :], gw[:, :])
            gmax8 = ap.tile([16, 8], f32, name="gmax8", tag="sm", bufs=24)
            gidx8 = ap.tile([16, 8], u32, name="gidx8", tag="sm", bufs=24)
            nc.vector.max_with_indices(gmax8[:, :], gidx8[:, :], gw[:, :])
            gidx8b = ap.tile([16, 8], u32, name="gidx8b", tag="sm", bufs=24)
            gmax8b = ap.tile([16, 8], f32, name="gmax8b", tag="sm", bufs=24)
            nc.vector.max_with_indices(gmax8b[:, :], gidx8b[:, :], glog[:, :])
            idx32 = ap.tile([16, 1], i32, name="idx32", tag="sm", bufs=24)
            nc.vector.tensor_copy(idx32[:, :], gidx8[:, 0:1])
            nc.sync.dma_start(idx_out[:, :], idx32[:, :])

            # expert mask row (1,16) via DRAM round-trip transpose
            idxf = ap.tile([16, 1], f32, name="idxf", tag="sm", bufs=24)
            nc.vector.tensor_copy(idxf[:, :], gidx8b[:, 0:1])
            drm = dr.tile([16, 1], f32, name="drm")
            nc.sync.dma_start(drm[:, :], idxf[:, :])
            idxrow = ap.tile([1, 16], f32, name="idxrow", tag="sm", bufs=24)
            nc.sync.dma_start(idxrow[:, :], drm[:, :].rearrange("b one -> one b"))
            maskrow = ap.tile([1, 16], f32, name="maskrow", tag="sm", bufs=24)
            nc.vector.tensor_scalar(maskrow[:, :], idxrow[:, :], ecmp_sb[:, :],
                                    None, op0=AL.is_equal)
            maskb = ap.tile([128, 16], f32, name="maskb", tag="sm", bufs=24)
            nc.gpsimd.partition_broadcast(maskb[:, :], maskrow[:, :])

            # bf16 copies of repT for the expert matmul
            repB = []
            for kt in range(NKT):
                rb_ = ap.tile([128, 16], bf16, name=f"repB_{kt}", tag="sm", bufs=24)
                nc.vector.tensor_copy(rb_[:, :], repT[kt][:, :])
                repB.append(rb_)
            ber_b = ap.tile([1, D], bf16, name="ber_b", tag="rowc", bufs=3)
            nc.vector.tensor_copy(ber_b[:, :], ber_sb[:, :])
            # ================= expert matmul (bf16) + AllReduce =================
            arin = dr.tile([D, 16], bf16, name="arin")
            arout = dr.tile([D, 16], bf16, name="arout", addr_space="Shared")
            for oi in range(NKT):
                we = wp.tile([128, NKT, 128], bf16, name=f"we_{oi}", tag="wt",
                             bufs=BUFS_WT)
                nc.sync.dma_start(we[:, :, :], io["weT"][oi])
                ps = ptile("ps_e")
                pieces = [(ber_b[:, oi * 128:oi * 128 + 128], ones16b[:, :])]
                pieces += [(we[:, kt, :], repB[kt][:, :]) for kt in range(NKT)]
                mm_group(ps[0:128, 0:16], pieces)
                contrib = ap.tile([128, 16], bf16, name="contrib", tag="sm", bufs=24)
                nc.vector.tensor_tensor(contrib[:, :], ps[0:128, 0:16],
                                        maskb[:, :], op=AL.mult)
                nc.sync.dma_start(arin[oi * 128:oi * 128 + 128, :], contrib[:, :])
            nc.gpsimd.collective_compute(
                "AllReduce", AL.add, replica_groups=RG,
                ins=[arin[:, :]], outs=[arout[:, :]])
            eo = []
            for kt in range(NKT):
                er = ap.tile([128, 16], bf16, name=f"eor_{kt}", tag="sm", bufs=24)
                nc.sync.dma_start(er[:, :], arout[kt * 128:kt * 128 + 128, :])
                eo.append(er)

            tail_scope.__exit__(None, None, None)
            # ================= head (vocab shard) =================
            head_scope = nc.named_scope("head")
            head_scope.__enter__()
            for vi in range(13):
                wh = wp.tile([128, NKT, 512], bf16, name=f"wh_{vi}",
                             tag="wvh", bufs=BUFS_WVH)
                # alternate DGE queues so head-weight streams overlap
                (nc.sync if vi % 2 == 0 else nc.gpsimd).dma_start(
                    wh[:, :, :], io["whT"][vi])
                bhc = ap.tile([1, 512], bf16, name="bhc", tag="bh", bufs=2)
                nc.sync.dma_start(bhc[:, :], io["bhr"][:, vi * 512:vi * 512 + 512])
                ps = ptile("ps_h")
                pieces = [(ones16b[:, :], bhc[:, :])]
                pieces += [(eo[kt][:, :], wh[:, kt, :]) for kt in range(NKT)]
                mm_group(ps[0:16, :], pieces)
                lg = ap.tile([16, 512], f32, name="lg", tag="lg", bufs=2)
                nc.vector.tensor_copy(lg[:, :], ps[0:16, :])
                nc.sync.dma_start(logits_s[:, vi * 512:vi * 512 + 512], lg[:, :])
            head_scope.__exit__(None, None, None)

    nc.compile()
    return nc


_NC_CACHE = None


def _get_nc():
    global _NC_CACHE
    if _NC_CACHE is None:
        _NC_CACHE = _build_nc()
    return _NC_CACHE


def _prep_inputs(inputs):
    g = {k: np.asarray(v, dtype=np.float32) for k, v in inputs.items()}
    x = g["x"]

    def tile4(w, no, nk, pk, po, dt=ml_dtypes.bfloat16):
        # w: (nk*pk, no*po) -> [no, pk, nk, po]: each [no] slice DMAs with
        # fully-contiguous per-partition rows
        return np.ascontiguousarray(
            w.reshape(nk, pk, no, po).transpose(2, 1, 0, 3).astype(dt))

    bfl = ml_dtypes.bfloat16
    com = {}
    for l in range(L):
        sprev = np.ones(D, np.float32) if l == 0 else g["ln2_s"][l - 1]
        bprev = np.zeros(D, np.float32) if l == 0 else g["ln2_b"][l - 1]
        Wqkv = g["Wqkv"][l]                      # (3D, D)
        beff = g["bqkv"][l] + Wqkv @ bprev       # (3D,)
        Weff = (Wqkv * sprev[None, :]).copy()
        Weff[:D] *= SCALE
        beff = beff.copy()
        beff[:D] *= SCALE
        WqkvT = np.ascontiguousarray(Weff.T)     # (D, 3D)
        com[f"wqkv{l}"] = tile4(WqkvT[:, :2 * D], 16, NKT, 128, 128)
        com[f"wv{l}"] = tile4(WqkvT[:, 2 * D:], 2, NKT, 128, 512)
        com[f"bqkc{l}"] = np.ascontiguousarray(beff[:2 * D].reshape(2 * D, 1))
        com[f"bvr{l}"] = np.ascontiguousarray(
            beff[2 * D:].reshape(1, D).astype(bfl))
        com[f"wo{l}"] = tile4(np.ascontiguousarray(g["Wo"][l].T), NKT, NKT, 128, 128)
        com[f"cbo{l}"] = np.ascontiguousarray(
            (g["bo"][l] + bprev).reshape(1, D).astype(bfl))
        s1 = g["ln1_s"][l]
        b1ln = g["ln1_b"][l]
        W1 = g["W1"][l]                          # (DFF, D)
        com[f"w1{l}"] = tile4(np.ascontiguousarray((W1 * s1[None, :]).T),
                              NFT, NKT, 128, 128)
        com[f"b1c{l}"] = np.ascontiguousarray((g["b1"][l] + W1 @ b1ln).reshape(DFF, 1))
        com[f"w2{l}"] = tile4(np.ascontiguousarray(g["W2"][l].T), NKT, NFT, 128, 128)
        com[f"cb2{l}"] = np.ascontiguousarray(
            (g["b2"][l] + b1ln).reshape(1, D).astype(bfl))
        com[f"sprev{l}"] = sprev.reshape(D, 1).copy()
        com[f"s1_{l}"] = s1.reshape(D, 1).copy()
    com["slmul"] = (g["ln2_s"][L - 1] / S).reshape(D, 1).copy()
    com["blast"] = g["ln2_b"][L - 1].reshape(D, 1).copy()
    com["wg"] = np.ascontiguousarray(g["Wg"].T)      # (D, E)
    com["bgr"] = g["bg"].reshape(1, E).copy()
    com["ones_col"] = np.ones((128, 1), np.float32)
    com["ones_row"] = np.ones((1, 512), bfl)
    com["ones16f"] = np.ones((1, 16), np.float32)

    WhT_pad = np.zeros((D, VPAD), np.float32)
    WhT_pad[:, :V] = g["Wh"].T
    bh_pad = np.zeros(VPAD, np.float32)
    bh_pad[:V] = g["bh"]

    in_maps = []
    for c in range(NC):
        m = dict(com)
        xs = x[:, BC * c:BC * (c + 1), :]            # (S, BC, D)
        xt_ = np.ascontiguousarray(
            xs.transpose(2, 1, 0).reshape(NKT, 128, T))
        m["xT"] = xt_
        m["xTb"] = xt_.astype(bfl)
        m["weT"] = tile4(np.ascontiguousarray(g["We"][c].T), NKT, NKT, 128, 128)
        m["ber"] = g["be"][c].reshape(1, D).copy()
        m["ecmp"] = np.full((1, 1), float(c), np.float32)
        whc = WhT_pad[:, c * VS:(c + 1) * VS]
        m["whT"] = tile4(whc, 13, NKT, 128, 512)
        m["bhr"] = np.ascontiguousarray(
            bh_pad[c * VS:(c + 1) * VS].reshape(1, VS).astype(bfl))
        in_maps.append(m)
    return in_maps


LAST_RESULTS = None


def kernel(**inputs):
    global LAST_RESULTS
    in_maps = _prep_inputs(inputs)
    nc = _get_nc()
    res = run_bass_kernel_spmd(
        nc, in_maps, core_ids=list(range(NC)),
        trace=os.environ.get("KERNEL_TRACE") == "1")
    LAST_RESULTS = res
    logits = np.concatenate([res.results[c]["logits_s"] for c in range(NC)],
                            axis=1)[:, :V]
    gating = res.results[0]["gw_out"]
    idx = res.results[0]["idx_out"].ravel().astype(np.int32)
    return np.ascontiguousarray(logits), np.ascontiguousarray(gating), idx
